# revision 1
# baseline (speedup 1.0000x reference)
"""Trainium2 Bass kernel for nn_GAT_9947144257800.

2-layer GAT, B=16, N=256. Data-parallel over B across 8 NeuronCores
(2 batches per core, no collectives).

Per core / batch / layer:
  hT = Wl^T @ xT + bl                  (PE, K-tiled PSUM accumulation)
  h  = transpose(hT)                   (PE transpose)
  sid2 = [Wa|Wa]^T @ hT                (doubled si^T, partitions (h, i-par))
  bcols = interleave([Wb|Wb]^T @ hT + ab1)   (per-j-pair bias columns)
  z_jp = relu(sid2 + bcols[:, jp])     (ACT activation / DVE tensor_scalar split)
  eT[2jp:2jp+2, :] = a2blockdiag^T @ z_jp    (PE, stationary a2)
  lg = lrelu(eT)*adjT + (adjT-1)*1e30  (ACT Lrelu + Pool tensor_tensor)
  att = exp(lg - max), D = sum         (flat softmax, unnormalized)
  x = (attT.T @ h) / D                 (PE aggregation, scaled on evacuation)
"""

import sys

if "/opt/trn_rl_repo" not in sys.path:
    sys.path.insert(0, "/opt/trn_rl_repo")

import numpy as np

B, N, IN_DIM, MEM, HID = 16, 256, 768, 300, 64
NCORES = 8
BLOC = B // NCORES  # batches per core
SLOPE = 0.01

# z-producer engine split pattern, cycled over j-pairs.
# D = DVE tensor_scalar (fused add+max), A = ACT activation(Relu, bias),
# P = Pool tensor_scalar. Weighted by modeled per-instr cost so every
# engine's total stays under the PE stream time (D~194ns, P~463ns, A~400ns).
Z_PATTERN = "ADPDDAPDDPDADPDDADPDDADPDPDADPDP"

_CACHE: dict = {}


def _build_nc(reps: int = 1):
    import concourse.mybir as mybir
    from concourse import bacc, tile

    f32 = mybir.dt.float32
    f32r = mybir.dt.float32r
    AL = mybir.AluOpType
    AF = mybir.ActivationFunctionType
    AX = mybir.AxisListType

    nc = bacc.Bacc()

    def dp(name, shape, is_out=False):
        return nc.declare_dram_parameter(name, list(shape), f32, isOutput=is_out)

    adj_d = dp("adj", (BLOC, N, N))
    feat_d = dp("feature", (BLOC, N, IN_DIM))
    w0_d = dp("w0", (IN_DIM, MEM))
    w1_d = dp("w1p", (384, MEM))
    wsia0_d = dp("wsia0", (IN_DIM, 128))
    wsjb0_d = dp("wsjb0", (IN_DIM, 128))
    wsia1_d = dp("wsia1", (384, 128))
    wsjb1_d = dp("wsjb1", (384, 128))
    bsi0_d = dp("bsi0", (128, 1))
    bsj0_d = dp("bsj0", (128, 1))
    bsi1_d = dp("bsi1", (128, 1))
    bsj1_d = dp("bsj1", (128, 1))
    b0r_d = dp("b0row", (1, MEM))
    b1r_d = dp("b1row", (1, MEM))
    ab2_d = dp("ab2col", (128, 1))
    a2m_d = dp("a2m", (128, 32, 64))
    id_d = dp("ident", (128, 128))
    o1r_d = dp("ones1x128", (1, 128))
    o1c_d = dp("ones128col", (128, 1))
    out_d = dp("out", (BLOC, N, MEM), is_out=True)

    KT0 = [(0, 128), (1, 128), (2, 128), (3, 128), (4, 128), (5, 128)]
    KT1 = [(0, 128), (1, 128), (2, 44)]
    MC = [(0, 0, 128), (1, 128, 128), (2, 256, 44)]  # (mc, m0, cp) chunks of 300

    with tile.TileContext(nc) as tc:
        import contextlib

        with contextlib.ExitStack() as ctx:
            wp = ctx.enter_context(tc.tile_pool(name="wconst", bufs=1))
            iop = ctx.enter_context(tc.tile_pool(name="io", bufs=2))
            adjp = ctx.enter_context(tc.tile_pool(name="adjp", bufs=2))
            xtp = ctx.enter_context(tc.tile_pool(name="xtp", bufs=2))
            work = ctx.enter_context(tc.tile_pool(name="work", bufs=3))
            zp = ctx.enter_context(tc.tile_pool(name="zp", bufs=28))
            smp = ctx.enter_context(tc.tile_pool(name="smp", bufs=2))
            ps_eT = ctx.enter_context(tc.tile_pool(name="ps_eT", bufs=4, space="PSUM"))
            ps_mm = ctx.enter_context(tc.tile_pool(name="ps_mm", bufs=3, space="PSUM"))
            ps_sm = ctx.enter_context(tc.tile_pool(name="ps_sm", bufs=1, space="PSUM"))
            if reps > 1:
                # timing variant: run the whole kernel body `reps` times on
                # device so per-iteration time can be extracted from wall
                # clock (no NTFF profiling available under this axon setup)
                ctx.enter_context(tc.For_i(0, reps, 1))

            # ---- persistent constants + feature loads.
            # DMA queue order = emission order: identity, feature(b0) and the
            # fused s-projection weights go first (they gate the first
            # z-phase); everything else fills in behind them.
            idsb = wp.tile([128, 128], f32)
            nc.sync.dma_start(idsb[:], id_d[:, :])
            fnat0 = iop.tile([128, 2, IN_DIM], f32, tag="fnat", name="fnat0")
            for it in range(2):
                for dh in range(2):
                    nc.sync.dma_start(
                        fnat0[:, it, dh * 384 : (dh + 1) * 384],
                        feat_d[0, it * 128 : (it + 1) * 128, dh * 384 : (dh + 1) * 384],
                    )
            wsia0sb = wp.tile([128, 6, 128], f32r)
            nc.sync.dma_start(wsia0sb[:], wsia0_d[:, :].rearrange("(k p) m -> p k m", p=128).bitcast(f32r))
            wsjb0sb = wp.tile([128, 6, 128], f32r)
            nc.sync.dma_start(wsjb0sb[:], wsjb0_d[:, :].rearrange("(k p) m -> p k m", p=128).bitcast(f32r))
            bsi0sb = wp.tile([128, 1], f32)
            nc.sync.dma_start(bsi0sb[:], bsi0_d[:, :])
            bsj0sb = wp.tile([128, 1], f32)
            nc.sync.dma_start(bsj0sb[:], bsj0_d[:, :])
            a2msb = wp.tile([128, 32, 64], f32r)
            nc.sync.dma_start(a2msb[:], a2m_d[:, :, :].bitcast(f32r))
            w0sb = wp.tile([128, 6, MEM], f32r)
            nc.sync.dma_start(w0sb[:], w0_d[:, :].rearrange("(k p) m -> p k m", p=128).bitcast(f32r))
            fnat1 = iop.tile([128, 2, IN_DIM], f32, tag="fnat", name="fnat1")
            for it in range(2):
                nc.sync.dma_start(
                    fnat1[:, it, :], feat_d[1, it * 128 : (it + 1) * 128, :]
                )
            w1sb = wp.tile([128, 3, MEM], f32r)
            nc.sync.dma_start(w1sb[:], w1_d[:, :].rearrange("(k p) m -> p k m", p=128).bitcast(f32r))
            wsia1sb = wp.tile([128, 3, 128], f32r)
            nc.sync.dma_start(wsia1sb[:], wsia1_d[:, :].rearrange("(k p) m -> p k m", p=128).bitcast(f32r))
            wsjb1sb = wp.tile([128, 3, 128], f32r)
            nc.sync.dma_start(wsjb1sb[:], wsjb1_d[:, :].rearrange("(k p) m -> p k m", p=128).bitcast(f32r))
            bsi1sb = wp.tile([128, 1], f32)
            nc.sync.dma_start(bsi1sb[:], bsi1_d[:, :])
            bsj1sb = wp.tile([128, 1], f32)
            nc.sync.dma_start(bsj1sb[:], bsj1_d[:, :])
            b0rsb = wp.tile([1, MEM], f32r)
            nc.sync.dma_start(b0rsb[:], b0r_d[:, :].bitcast(f32r))
            b1rsb = wp.tile([1, MEM], f32r)
            nc.sync.dma_start(b1rsb[:], b1r_d[:, :].bitcast(f32r))
            o1rrsb = wp.tile([1, 128], f32r)
            nc.sync.dma_start(o1rrsb[:], o1r_d[:, :].bitcast(f32r))
            ab2sb = wp.tile([128, 1], f32)
            nc.sync.dma_start(ab2sb[:], ab2_d[:, :])
            o1rsb = wp.tile([1, 128], f32)
            nc.sync.dma_start(o1rsb[:], o1r_d[:, :])
            o1csb = wp.tile([128, 1], f32)
            nc.sync.dma_start(o1csb[:], o1c_d[:, :])
            zcol = wp.tile([128, 1], f32)
            nc.vector.memset(zcol[:], 0.0)

            xT0, adjT, negm = {}, {}, {}

            def adj_prep(b):
                anat = iop.tile([128, 2, N], f32, tag="anat")
                nc.sync.dma_start(
                    anat[:], adj_d[b, :, :].rearrange("(i p) j -> p i j", p=128)
                )
                aT = adjp.tile([128, 2, N], f32, tag="aT")
                for it in range(2):
                    for jt in range(2):
                        pt = ps_mm.tile([128, MEM], f32, tag="pt")
                        nc.tensor.transpose(
                            pt[:, 0:128],
                            anat[:, it, jt * 128 : (jt + 1) * 128],
                            idsb[:],
                        )
                        dst = aT[:, jt, it * 128 : (it + 1) * 128]
                        if jt == 0:
                            nc.scalar.copy(dst, pt[:, 0:128])
                        else:
                            nc.vector.tensor_copy(dst, pt[:, 0:128])
                adjT[b] = aT

            def _wsel(layer):
                if layer == 0:
                    return w0sb, wsia0sb, wsjb0sb, bsi0sb, bsj0sb, b0rsb
                return w1sb, wsia1sb, wsjb1sb, bsi1sb, bsj1sb, b1rsb

            def prep_s_sid(layer, xTb, ktiles):
                """si doubled, straight from xT via host-fused Wl@Wa."""
                wn, wsia, wsjb, bsi, bsj, brow = _wsel(layer)
                nkt = len(ktiles)
                sid2 = work.tile([128, N], f32, tag="sid2")
                pts = ps_mm.tile([128, MEM], f32, tag="pt")
                for kt, kr in ktiles:
                    nc.tensor.matmul(
                        pts[0:128, 0:N],
                        wsia[0:kr, kt, :],
                        xTb[0:kr, kt, :],
                        start=(kt == 0),
                        stop=(kt == nkt - 1),
                    )
                nc.scalar.activation(
                    sid2[:], pts[0:128, 0:N], AF.Identity, bias=bsi[:, 0:1]
                )
                return sid2

            def prep_s_bcols(layer, xTb, ktiles):
                """sj doubled -> interleaved per-j-pair bias columns (+ab1)."""
                wn, wsia, wsjb, bsi, bsj, brow = _wsel(layer)
                nkt = len(ktiles)
                bcols = work.tile([128, 128], f32, tag="bcols")
                ptj = ps_mm.tile([128, MEM], f32, tag="pt")
                for kt, kr in ktiles:
                    nc.tensor.matmul(
                        ptj[0:128, 0:N],
                        wsjb[0:kr, kt, :],
                        xTb[0:kr, kt, :],
                        start=(kt == 0),
                        stop=(kt == nkt - 1),
                    )
                lo = ptj[0:64, 0:N].rearrange("p (j two) -> p j two", two=2)
                hi = ptj[64:128, 0:N].rearrange("p (j two) -> p j two", two=2)
                nc.scalar.activation(
                    bcols[0:64, :], lo[:, :, 0], AF.Identity, bias=bsj[0:64, 0:1]
                )
                nc.scalar.activation(
                    bcols[64:128, :], hi[:, :, 1], AF.Identity, bias=bsj[64:128, 0:1]
                )
                return bcols

            def prep_h(layer, xTb, ktiles):
                """h natural [i, m] = x @ Wl + bl; bias applied as a rank-1
                ones x b_row matmul opening each PSUM accumulation group.
                Only needed at aggregation time, so off the critical path."""
                wn, wsia, wsjb, bsi, bsj, brow = _wsel(layer)
                nkt = len(ktiles)
                h = work.tile([128, 2, MEM], f32r, tag="h")
                for it in range(2):
                    pt = ps_mm.tile([128, MEM], f32, tag="pt")
                    nc.tensor.matmul(
                        pt[0:128, 0:MEM],
                        o1rrsb[0:1, :],
                        brow[0:1, :],
                        start=True,
                        stop=False,
                        skip_group_check=True,
                    )
                    for kt, kr in ktiles:
                        nc.tensor.matmul(
                            pt[0:128, 0:MEM],
                            xTb[0:kr, kt, it * 128 : (it + 1) * 128],
                            wn[0:kr, kt, :],
                            start=False,
                            stop=(kt == nkt - 1),
                            skip_group_check=True,
                        )
                    nc.scalar.copy(h[:, it, :], pt[0:128, 0:MEM])
                return h

            Z_BREAKS = (8, 24, 40, 56, 72, 88, 104, 120)

            def z_phase(sid2, bcols, fillers=(), eTs=None):
                """128 producer+reduce pairs; `fillers` are closures emitted at
                fixed j-pair breakpoints so their engine work interleaves with
                this phase's producer stream instead of serializing after it."""
                if eTs is None:
                    eTs = [
                        ps_eT.tile([64, 2, N], f32, tag="eT", name=f"eT{i}")
                        for i in range(2)
                    ]
                np_ = len(Z_PATTERN)
                fills = list(fillers)
                res = []
                for jp in range(128):
                    for k, bp in enumerate(Z_BREAKS):
                        if jp == bp and k < len(fills):
                            res.append(fills[k]())
                    z = zp.tile([128, N], f32r, tag="z")
                    eng = Z_PATTERN[jp % np_]
                    bc = bcols[:, jp : jp + 1]
                    if eng == "A":
                        nc.scalar.activation(z[:], sid2[:], AF.Relu, bias=bc)
                    elif eng == "D":
                        nc.vector.tensor_scalar(z[:], sid2[:], bc, 0.0, AL.add, AL.max)
                    else:
                        nc.gpsimd.tensor_scalar(z[:], sid2[:], bc, 0.0, AL.add, AL.max)
                    # e^T rows land in 64-row group g of tile jt, packed
                    # side-by-side in one PSUM bank (f32r matmuls may only
                    # target PSUM partition base 0). 32 MMs accumulate per
                    # group; a2m is zero except this j-pair's two columns.
                    jt, rp = divmod(jp, 64)
                    g, v = divmod(rp, 32)
                    nc.tensor.matmul(
                        eTs[jt][0:64, g, 0:N],
                        a2msb[:, v, :],
                        z[:],
                        start=(v == 0),
                        stop=(v == 31),
                    )
                return eTs, res

            def sm_s1_tile(lg, e1, eT, jt):
                # leaky-relu as max(x, SLOPE*x) using only Identity on ACT
                # (keeps every ACT func inside the exp_and_friends table set --
                # Lrelu would force a ~1.3us LoadActFuncSet per switch).
                # Masking happens later as exp(x)*adj, so no -1e30 logits.
                nc.scalar.activation(
                    e1[0:64, jt, :], eT[0:64, 0, 0:N], AF.Identity,
                    bias=ab2sb[0:64, 0:1],
                )
                nc.scalar.activation(
                    e1[64:128, jt, :], eT[0:64, 1, 0:N], AF.Identity,
                    bias=ab2sb[64:128, 0:1],
                )
                nc.gpsimd.tensor_scalar_mul(lg[:, jt, :], e1[:, jt, :], SLOPE)
                nc.vector.tensor_tensor(
                    lg[:, jt, :], lg[:, jt, :], e1[:, jt, :], AL.max
                )

            def sm_s2a(lg):
                # flat-softmax global max (negated, broadcast-ready)
                mx = smp.tile([128, 1], f32, tag="mx")
                nc.vector.tensor_reduce(mx[:, 0:1], lg[:], AX.XY, AL.max)
                ptm = ps_sm.tile([128, 128], f32, tag="st")
                nc.tensor.transpose(ptm[0:1, 0:128], mx[:, 0:1], idsb[:])
                m1 = smp.tile([1, 1], f32, tag="m1")
                nc.vector.tensor_reduce(m1[0:1, 0:1], ptm[0:1, 0:128], AX.X, AL.max)
                m1n = smp.tile([1, 1], f32, tag="m1n")
                nc.vector.tensor_scalar_mul(m1n[0:1, 0:1], m1[0:1, 0:1], -1.0)
                return m1n

            def sm_s2b(lg, m1n, aT):
                # att = exp(lg - max) * adjT ; D = sum(att); 1/D broadcast
                ptb = ps_sm.tile([128, 128], f32, tag="st")
                nc.tensor.matmul(
                    ptb[0:128, 0:1], o1rsb[0:1, :], m1n[0:1, 0:1],
                    start=True, stop=True,
                )
                nmax = smp.tile([128, 1], f32, tag="nmax")
                nc.scalar.copy(nmax[:], ptb[0:128, 0:1])
                attr = smp.tile([128, 2, N], f32, tag="attr")
                nc.scalar.activation(attr[:], lg[:], AF.Exp, bias=nmax[:, 0:1])
                att = smp.tile([128, 2, N], f32r, tag="att")
                nc.vector.tensor_tensor(att[:], attr[:], aT[:], AL.mult)
                rows = smp.tile([128, 1], f32, tag="rows")
                nc.vector.tensor_reduce(rows[:, 0:1], att[:], AX.XY, AL.add)
                ptd = ps_sm.tile([128, 128], f32, tag="st")
                nc.tensor.matmul(
                    ptd[0:1, 0:1], rows[:, 0:1], o1csb[:, 0:1], start=True, stop=True
                )
                dr = smp.tile([1, 1], f32, tag="dr")
                nc.vector.reciprocal(dr[0:1, 0:1], ptd[0:1, 0:1])
                ptb2 = ps_sm.tile([128, 128], f32, tag="st")
                nc.tensor.matmul(
                    ptb2[0:128, 0:1], o1rsb[0:1, :], dr[0:1, 0:1],
                    start=True, stop=True,
                )
                dscale = smp.tile([128, 1], f32, tag="dscale")
                nc.scalar.copy(dscale[:], ptb2[0:128, 0:1])
                return att, dscale

            def agg_l0(h, att, dscale):
                x1T = xtp.tile([128, 3, N], f32r, tag="x1T")
                for mc, m0, cp in MC:
                    pt = ps_mm.tile([128, MEM], f32, tag="pt")
                    for jt in range(2):
                        nc.tensor.matmul(
                            pt[0:cp, 0:N],
                            h[:, jt, m0 : m0 + cp],
                            att[:, jt, :],
                            start=(jt == 0),
                            stop=(jt == 1),
                        )
                    nc.scalar.activation(
                        x1T[0:cp, mc, :], pt[0:cp, 0:N], AF.Identity,
                        bias=zcol[0:cp, 0:1], scale=dscale[0:cp, 0:1],
                    )
                return x1T

            def agg_l1(b, h, att, dscale):
                for it in range(2):
                    pt = ps_mm.tile([128, MEM], f32, tag="pt")
                    for jt in range(2):
                        nc.tensor.matmul(
                            pt[0:128, 0:MEM],
                            att[:, jt, it * 128 : (it + 1) * 128],
                            h[:, jt, :],
                            start=(jt == 0),
                            stop=(jt == 1),
                        )
                    osb = smp.tile([128, MEM], f32, tag="osb")
                    nc.scalar.activation(
                        osb[:], pt[0:128, 0:MEM], AF.Identity,
                        bias=zcol[:, 0:1], scale=dscale[:, 0:1],
                    )
                    nc.sync.dma_start(out_d[b, it * 128 : (it + 1) * 128, :], osb[:])

            # ---- schedule: four z-phases back to back; every other piece
            # of work (softmax, aggregation, next prep, batch-1 input prep)
            # is a small closure emitted at a breakpoint inside some phase so
            # its engine ops interleave with that phase's producer stream.
            S, Bc, H, X = {}, {}, {}, {}

            def featT(b, it):
                fnat = fnat0 if b == 0 else fnat1
                xTb = xT0[b]
                for kt in range(6):
                    pt = ps_mm.tile([128, MEM], f32, tag="pt")
                    nc.tensor.transpose(
                        pt[:, 0:128],
                        fnat[:, it, kt * 128 : (kt + 1) * 128],
                        idsb[:],
                    )
                    dst = xTb[:, kt, it * 128 : (it + 1) * 128]
                    if kt % 2 == 0:
                        nc.scalar.copy(dst, pt[:, 0:128])
                    else:
                        nc.vector.tensor_copy(dst, pt[:, 0:128])

            xT0[0] = xtp.tile([128, 6, N], f32r, tag="xT0", name="xT0_0")
            xT0[1] = xtp.tile([128, 6, N], f32r, tag="xT0", name="xT0_1")
            featT(0, 0)
            featT(0, 1)
            S[(0, 0)] = prep_s_sid(0, xT0[0], KT0)
            Bc[(0, 0)] = prep_s_bcols(0, xT0[0], KT0)

            eT00, _ = z_phase(
                S[(0, 0)],
                Bc[(0, 0)],
                fillers=(
                    lambda: featT(1, 0),
                    lambda: featT(1, 1),
                    lambda: S.__setitem__((1, 0), prep_s_sid(0, xT0[1], KT0)),
                    lambda: Bc.__setitem__((1, 0), prep_s_bcols(0, xT0[1], KT0)),
                    lambda: adj_prep(0),
                    lambda: adj_prep(1),
                    lambda: H.__setitem__((0, 0), prep_h(0, xT0[0], KT0)),
                    lambda: H.__setitem__((1, 0), prep_h(0, xT0[1], KT0)),
                ),
            )

            def mk_fillers(eTs, b, layer, nxt):
                """8 fillers: s1(tile0), s1(tile1), max, exp+denom,
                agg, next prep_s(sid), next prep_s(bcols), next prep_h."""
                box = {}
                lg = smp.tile([128, 2, N], f32, tag="lg", name=f"lg{b}{layer}")
                e1 = smp.tile([128, 2, N], f32, tag="e1", name=f"e1{b}{layer}")

                def f1():
                    sm_s1_tile(lg, e1, eTs[0], 0)

                def f2():
                    sm_s1_tile(lg, e1, eTs[1], 1)

                def f3():
                    box["m1n"] = sm_s2a(lg)

                def f4():
                    box["att"], box["ds"] = sm_s2b(lg, box["m1n"], adjT[b])

                def f5():
                    if layer == 0:
                        X[b] = agg_l0(H[(b, 0)], box["att"], box["ds"])
                    else:
                        agg_l1(b, H[(b, 1)], box["att"], box["ds"])

                def f6():
                    if nxt:
                        S[(b, 1)] = prep_s_sid(1, X[b], KT1)

                def f7():
                    if nxt:
                        Bc[(b, 1)] = prep_s_bcols(1, X[b], KT1)

                def f8():
                    if nxt:
                        H[(b, 1)] = prep_h(1, X[b], KT1)

                return (f1, f2, f3, f4, f5, f6, f7, f8)

            eT10, _ = z_phase(
                S[(1, 0)], Bc[(1, 0)], fillers=mk_fillers(eT00, 0, 0, True)
            )
            eT01, _ = z_phase(
                S[(0, 1)], Bc[(0, 1)], fillers=mk_fillers(eT10, 1, 0, True)
            )
            # last phase: its own eT[0] is complete after jp=64, so the final
            # softmax's first-tile stage1 runs inside the phase (jp>=88 slot).
            eT11 = [
                ps_eT.tile([64, 2, N], f32, tag="eT", name=f"eT11_{i}")
                for i in range(2)
            ]
            lg11 = smp.tile([128, 2, N], f32, tag="lg", name="lg11")
            e111 = smp.tile([128, 2, N], f32, tag="e1", name="e111")
            f1, f2, f3, f4, f5, _, _, _ = mk_fillers(eT01, 0, 1, False)
            z_phase(
                S[(1, 1)],
                Bc[(1, 1)],
                fillers=(
                    f1, f2, f3, f4, f5,
                    lambda: None,
                    lambda: sm_s1_tile(lg11, e111, eT11[0], 0),
                    lambda: None,
                ),
                eTs=eT11,
            )
            sm_s1_tile(lg11, e111, eT11[1], 1)
            m1n11 = sm_s2a(lg11)
            att11, ds11 = sm_s2b(lg11, m1n11, adjT[1])
            agg_l1(1, H[(1, 1)], att11, ds11)

    nc.compile()
    return nc


def _host_params(W0, b0, W1, b1, A1, ab1, A2, ab2):
    f = np.float32
    d = np.float64
    Wa, Wb = np.asarray(A1[:MEM], d), np.asarray(A1[MEM:], d)
    a2 = np.asarray(A2, d)[:, 0]
    W0 = np.asarray(W0, d)
    W1 = np.asarray(W1, d)
    b0 = np.asarray(b0, d)
    b1 = np.asarray(b1, d)
    ab1 = np.asarray(ab1, d)

    def pad_rows(x, rows):
        out = np.zeros((rows,) + x.shape[1:], f)
        out[: x.shape[0]] = x
        return out

    def dbl(x):  # [K, 64] -> [K, 128] doubled columns
        return np.concatenate([x, x], axis=1)

    def dupcol(v):  # [64] -> [128, 1]
        return np.concatenate([v, v]).astype(f)[:, None].copy()

    w1p = pad_rows(W1.astype(f), 384)
    ab2v = float(np.asarray(ab2, f).reshape(-1)[0])
    a2m = np.zeros((128, 32, 64), f)
    for v in range(32):
        a2m[0:64, v, 2 * v] = a2
        a2m[64:128, v, 2 * v + 1] = a2
    return dict(
        w0=np.ascontiguousarray(W0, f),
        w1p=w1p,
        wsia0=np.ascontiguousarray(dbl(W0 @ Wa), f),
        wsjb0=np.ascontiguousarray(dbl(W0 @ Wb), f),
        wsia1=pad_rows(dbl(W1 @ Wa).astype(f), 384),
        wsjb1=pad_rows(dbl(W1 @ Wb).astype(f), 384),
        bsi0=dupcol(b0 @ Wa),
        bsj0=dupcol(b0 @ Wb + ab1),
        bsi1=dupcol(b1 @ Wa),
        bsj1=dupcol(b1 @ Wb + ab1),
        b0row=np.ascontiguousarray(b0.astype(f)[None, :]),
        b1row=np.ascontiguousarray(b1.astype(f)[None, :]),
        ab2col=np.full((128, 1), ab2v, f),
        a2m=a2m,
        ident=np.eye(128, dtype=f),
        ones1x128=np.ones((1, 128), f),
        ones128col=np.ones((128, 1), f),
    )


def get_nc(reps: int = 1):
    key = f"nc{reps}"
    if key not in _CACHE:
        _CACHE[key] = _build_nc(reps)
    return _CACHE[key]


def kernel(adj, feature, W0, b0, W1, b1, A1, ab1, A2, ab2):
    from concourse.bass_utils import run_bass_kernel_spmd

    nc = get_nc()
    params = _host_params(W0, b0, W1, b1, A1, ab1, A2, ab2)
    f = np.float32
    adj = np.ascontiguousarray(adj, f)
    feature = np.ascontiguousarray(feature, f)
    in_maps = []
    for c in range(NCORES):
        m = dict(params)
        m["adj"] = adj[c * BLOC : (c + 1) * BLOC]
        m["feature"] = feature[c * BLOC : (c + 1) * BLOC]
        in_maps.append(m)
    r = run_bass_kernel_spmd(nc, in_maps, list(range(NCORES)))
    out = np.concatenate([r.results[c]["out"] for c in range(NCORES)], axis=0)
    return out.astype(np.float32)

